# revision 65
# baseline (speedup 1.0000x reference)
"""Megatron-style MHA on 8 Trainium2 NeuronCores.

Problem: B=4, T=2048, C=1024, 16 heads, head_dim=64, causal attention, fp32.
  qkv = x @ Wqkv^T; attention per head; out = attn @ Wproj^T

Sharding (tensor-parallel over heads + AllToAll reshard):
  - Core c owns heads {2c, 2c+1}: computes Q/K/V (column-parallel Wqkv slice)
    and causal attention for those heads over all batches/positions.
  - Attention outputs are resharded with four per-batch AllToAll collectives
    so each core ends up with the full 1024 attn features for 1/8 of the t
    positions; every collective overlaps compute (proj for batch b is
    deferred two batches, pushing its matmuls into the Act-bound tail of
    the attention pipeline where the PE would otherwise idle).
  - Each core then applies the full Wproj to its t-slices (data-parallel), so
    no reduction collective is needed.

Precision/perf: the two big GEMMs (QKV, proj) use RESIDUAL-FP8 DoubleRow:
each operand a is scaled into e4m3 range and split a*2^s = hi + lo; the
product accumulates hi*hi + hi*lo + lo*hi in fp32 PSUM via fp8 DoubleRow
matmuls (0.5 PE cycles/row, contraction 256/instruction): 6 moving passes
per 8x128 contraction vs 8 for bf16 (1.33x) with BETTER-than-bf16 accuracy
(dropped lo*lo term ~0.2%). x/weight splits happen on the host; the attn
output is split on-chip (copy + subtract on DVE) and both fp8 halves ride
the same-sized AllToAll payload a bf16 tensor would. All descales fold into
existing constants (exp scale, denominator ones column, output descale).
Scores/attn@V stay bf16: plain fp8 fails the 2e-2 gate (~4.5e-2 measured)
and head_dim=64 makes the DoubleRow d-split layout unprofitable for scores.
Attention weights/q/k at rest are bf16; matmuls accumulate in fp32 PSUM.

Pipeline: the PE executes in order, and exp() makes the Activation engine
the throughput limit of late attention q-chunks (~151us exp vs ~177us PE).
Independent PE work is interleaved between the scores and attn@V of
q-chunks 4..7: the NEXT batch's qkv chunks (or, for the last batch, the
deferred proj quarters). Per-qc a2a payload DMAs issue from the gpsimd
SWDGE queue so the SP HWDGE queue never head-of-line blocks bulk x/recv/y
traffic; x streams chunk-major so batch-0 compute starts ~1.5us in; y
writes go out in fine-grained per-2-block DMAs to overlap the drain.

Layouts are chosen so no operand ever needs a transpose except the final
attention output:
  - q/k kept as [feature, t] (contraction dim on partitions for scores).
  - v produced directly as [t, feature] tiles (stationary = x-tile,
    moving = Wv slice), which is what attn@V consumes -- no PE transpose
    of V and no staging copy.
  - attn@V runs "flipped": stationary = exp(scores) [128 keys, 128 q],
    moving = v (+ ones column for the softmax denominator) [128 keys, 65].
    Output is [q, 65] so the PE streams only 65 columns per key-tile
    instead of 128+ -- half the cycles of the unflipped form.
  - The softmax divide is a per-partition reciprocal + tensor_scalar_mul
    (denominator lands on the q partition), then one bf16 PE transpose per
    128-q tile flips the normalized output to [feature, t] for the AllToAll.

Softmax: scores are O(1) (inputs are unit-scale gaussians), so exp() without
max-subtraction is safe. exp reads fp32 PSUM scores and writes bf16.
"""

import numpy as np
import ml_dtypes

import concourse.mybir as mybir
import concourse.tile as tile
from concourse import bacc
from concourse.bass_utils import run_bass_kernel_spmd

B, T, C, H, D = 4, 2048, 1024, 16, 64
NCORE = 8
HPC = H // NCORE  # 2 heads per core
BT = B * T
QC = 256  # q-chunk width for attention (one a2a chunk)
NQC = T // QC  # 8 q-chunks per batch
NKT = T // 128  # 16 k-tiles per batch
TCH = 512  # t-chunk width for the qkv projection
QW = T // NCORE  # 256: per-core t-slice of one batch

F32 = mybir.dt.float32
BF16 = mybir.dt.bfloat16
FP8 = mybir.dt.float8e4
DR = mybir.MatmulPerfMode.DoubleRow
# residual-fp8 scale exponents: x*2^SX and w*2^SW are split into e4m3
# hi+lo pairs on the host; q/k stay scaled in SBUF and the descale
# 2^-2(SX+SW) folds into the exp scale; v descales at its PSUM copy.
SX, SW = 4, 8
QKSCALE = 2.0 ** (SX + SW)
EXP = mybir.ActivationFunctionType.Exp


def build_nc(sim_mode: bool = False, max_stage: int = 99, debug_attn: bool = False):
    # sim_mode: skip collectives (TimelineSim is single-core) — timing study only
    # max_stage: emit only the first N stages (timing bisection in sim_mode)
    nc = bacc.Bacc("TRN2", target_bir_lowering=False, debug=False, num_devices=NCORE)

    # x8: residual-fp8 split of x*2^SX, laid out per ct-pair for DoubleRow:
    # [partition, batch, ct-pair, ct-in-pair, {lo,hi}, t]
    x8 = nc.dram_tensor("x8", [128, B, 4, 2, 2, T], FP8, kind="ExternalInput")
    # wq8: per-core wqkv slice *2^SW split hi/lo: [p, ct-pair, ci, {hi,lo}, 384]
    wq8 = nc.dram_tensor("wq8", [128, 4, 2, 2, 3 * 128], FP8, kind="ExternalInput")
    # wp8: Wproj^T *2^SW residual split: [p, ct-pair, ci, {hi,lo}, 1024]
    wp8 = nc.dram_tensor("wp8", [128, 4, 2, 2, C], FP8, kind="ExternalInput")
    identb = nc.dram_tensor("identb", [128, 128], BF16, kind="ExternalInput")
    trib = nc.dram_tensor("trib", [128, 128], BF16, kind="ExternalInput")
    yT = nc.dram_tensor("yT", [C, B * QW], F32, kind="ExternalOutput")

    # AllToAll buffers, one per batch:
    # [8 chunks, 128 feat (2 heads), {lo,hi} residual-fp8 of attn*2^SX, 256 t]
    a2a_in = [
        nc.dram_tensor(f"a2a_in{i}", [NCORE, 128, 2, QW], FP8, kind="Internal")
        for i in range(B)
    ]
    dbg = (
        nc.dram_tensor("dbg", [NQC, 128, 2, QW], FP8, kind="ExternalOutput")
        if debug_attn
        else None
    )
    a2a_out = [
        nc.dram_tensor(f"a2a_out{i}", [NCORE, 128, 2, QW], FP8, kind="Internal")
        for i in range(B)
    ]
    groups = [list(range(NCORE))]

    with tile.TileContext(nc) as tc:
        with (
            tc.tile_pool(name="const", bufs=1) as constp,
            tc.tile_pool(name="xt", bufs=9) as xtp,
            tc.tile_pool(name="kt", bufs=3) as ktp,
            tc.tile_pool(name="qt", bufs=3) as qtp,
            tc.tile_pool(name="vaug", bufs=3) as vaugp,
            tc.tile_pool(name="pt", bufs=16) as ptp,
            tc.tile_pool(name="zn", bufs=8) as znp,
            tc.tile_pool(name="rec", bufs=16) as recp,
            tc.tile_pool(name="stage", bufs=8) as stagep,
            tc.tile_pool(name="recv", bufs=4) as recvp,
            tc.tile_pool(name="ys", bufs=2) as ysp,
            tc.tile_pool(name="pss", bufs=2, space="PSUM") as pss,
            tc.tile_pool(name="pso", bufs=1, space="PSUM") as pso,
            tc.tile_pool(name="pst", bufs=1, space="PSUM") as pst,
            tc.tile_pool(name="psm", bufs=2, space="PSUM") as psm,
        ):
            # ---- constants ----
            # wqkv loads are interleaved with the first x chunk (see qkv_batch)
            wqkv_sb = constp.tile([128, 4, 2, 2, 3 * 128], FP8, tag="wqkv")
            wproj_sb = constp.tile([128, 4, 2, 2, C], FP8, tag="wproj")

            def load_wproj():
                # deferred: wproj is first needed by proj(0), which runs after
                # attention of batch 1 — keep it off the startup critical path
                nc.sync.dma_start(wproj_sb[:], wp8[:])

            ident_sb = constp.tile([128, 128], BF16, tag="ident")
            tri_sb = constp.tile([128, 128], BF16, tag="tri")

            def load_consts():
                # issued after the first batch's x/wqkv DMAs: first needed by
                # attention of batch 0, ~20us in (ident loads earlier: the
                # v transposes in qkv(0) need it ~6us in)
                nc.sync.dma_start(tri_sb[:], trib[:])
                # Pre-zero score PSUM slots: diagonal tiles only write the
                # causal column range, and exp() reads the full tile; stale
                # bits from uninitialized PSUM could be NaN/Inf otherwise.
                for _ in range(2):
                    z = pss.tile([128, 4 * QC], F32, tag="s")
                    nc.vector.memset(z[:], 0.0)

            def load_x(b):
                # 4 ct-pair tiles per batch: [128, ci, {lo,hi}, T] fp8
                xts = []
                if b == 0:
                    # PE clock warm-up: the HAM gate runs the PE at half rate
                    # until ~3us of sustained activity. Burn dummy matmuls on
                    # a memset tile during the initial DMA wait so the real
                    # qkv chain starts at full clock.
                    warm = constp.tile([128, TCH], BF16, tag="warm")
                    nc.gpsimd.memset(warm[:], 0.0)
                    pswarm = psm.tile([128, TCH], F32, tag="m", name="pswarm")
                    for _ in range(8):
                        nc.tensor.matmul(
                            pswarm[:], warm[:, 0:128], warm[:], start=True, stop=True
                        )
                    # first batch: wqkv halves + per-cp 1024-wide x halves,
                    # ordered so the first matmul chain starts early and the
                    # PE stays fed while the rest streams
                    xts = [
                        xtp.tile([128, 2, 2, T], FP8, tag="xt", name=f"xt0_{cp}")
                        for cp in range(4)
                    ]
                    nc.sync.dma_start(wqkv_sb[:, 0:2], wq8[:, 0:2])
                    nc.sync.dma_start(wqkv_sb[:, 2:4], wq8[:, 2:4])
                    nc.sync.dma_start(ident_sb[:], identb[:])
                    # stream x chunk-major so the first 512-col qkv chain can
                    # start after ~1.5us and compute stays ahead of the DMAs
                    for tch in range(T // TCH):
                        sl = slice(tch * TCH, (tch + 1) * TCH)
                        for cp in range(4):
                            nc.sync.dma_start(
                                xts[cp][:, :, :, sl], x8[:, 0, cp, :, :, sl]
                            )
                        if tch == 0:
                            load_consts()
                else:
                    for cp in range(4):
                        xt_tile = xtp.tile([128, 2, 2, T], FP8, tag="xt")
                        nc.sync.dma_start(xt_tile[:], x8[:, b, cp])
                        xts.append(xt_tile)
                return xts

            def qkv_alloc():
                """qt/kt: [128 (2 heads x 64d), 2048] bf16. V -> va [t, d] tiles."""
                kt_t = ktp.tile([128, T], BF16, tag="kt")
                qt_t = qtp.tile([128, T], BF16, tag="qt")
                va_t = vaugp.tile([128, NKT, 130], BF16, tag="vaug")
                # denominator columns at 64 and 129 of each [*, kt, :] slice;
                # value 2^-SX makes the normalize emit attn*2^SX, the scale
                # the residual-fp8 proj input wants
                nc.vector.memset(va_t[:, :, 64:130:65], 1.0 / (1 << SX))
                return qt_t, kt_t, va_t

            def qkv_chunk(tiles, xts, tch):
                qt_t, kt_t, va_t = tiles
                if True:
                    sl = slice(tch * TCH, (tch + 1) * TCH)
                    for o in range(2):  # q, k feature blocks (128 each)
                        ps = psm.tile([128, TCH], F32, tag="m", name="psqk")
                        fs = slice(o * 128, (o + 1) * 128)
                        for cp in range(4):  # main: w-hi pair x x-hi pair
                            nc.tensor.matmul(
                                ps[:],
                                wqkv_sb[:, cp, 0:2, 0, fs],
                                xts[cp][:, 0:2, 1, sl],
                                start=(cp == 0),
                                stop=False,
                                perf_mode=DR,
                            )
                        for cp in range(4):  # corr: (wh,wl) x (xl,xh)
                            for ci in range(2):
                                nc.tensor.matmul(
                                    ps[:],
                                    wqkv_sb[:, cp, ci, 0:2, fs],
                                    xts[cp][:, ci, 0:2, sl],
                                    start=False,
                                    stop=(cp == 3 and ci == 1),
                                    perf_mode=DR,
                                )
                        # q/k stay scaled by 2^(SX+SW); descale folds into exp
                        dst = qt_t if o == 0 else kt_t
                        nc.vector.tensor_copy(dst[:, sl], ps[:])
                    for tt in range(TCH // 128):
                        # v, flipped: out [128 t, 128 d] straight into va
                        kti = tch * (TCH // 128) + tt
                        t0 = tch * TCH + tt * 128
                        psv = psm.tile([128, 128], F32, tag="m", name="psv")
                        vfs = slice(256, 384)
                        for cp in range(4):
                            nc.tensor.matmul(
                                psv[:],
                                xts[cp][:, 0:2, 1, t0 : t0 + 128],
                                wqkv_sb[:, cp, 0:2, 0, vfs],
                                start=(cp == 0),
                                stop=False,
                                perf_mode=DR,
                            )
                        for cp in range(4):
                            for ci in range(2):
                                nc.tensor.matmul(
                                    psv[:],
                                    xts[cp][:, ci, 0:2, t0 : t0 + 128],
                                    wqkv_sb[:, cp, ci, 0:2, vfs],
                                    start=False,
                                    stop=(cp == 3 and ci == 1),
                                    perf_mode=DR,
                                )
                        dst = va_t[:, kti].rearrange("p (two s) -> p two s", s=65)[
                            :, :, 0:64
                        ]
                        nc.vector.tensor_scalar_mul(
                            dst,
                            psv[:].rearrange("p (two s) -> p two s", s=64),
                            1.0 / QKSCALE,
                        )

            def attn_batch(b, qt_t, kt_t, va_t, fillers=()):
                for qc in range(NQC):
                    pts = []
                    for pi in range(qc + 1):  # kt pairs (2pi, 2pi+1)
                        psS = pss.tile([128, 4 * QC], F32, tag="s", name="psS")
                        for ii in range(2):
                            kt = 2 * pi + ii
                            lo = max(0, 128 * kt - QC * qc)
                            for hl in range(HPC):
                                qtr = 2 * hl + ii
                                nc.tensor.matmul(
                                    psS[:, QC * qtr + lo : QC * (qtr + 1)],
                                    kt_t[
                                        64 * hl : 64 * hl + 64,
                                        128 * kt : 128 * (kt + 1),
                                    ],
                                    qt_t[
                                        64 * hl : 64 * hl + 64,
                                        QC * qc + lo : QC * (qc + 1),
                                    ],
                                    start=True,
                                    stop=True,
                                )
                        pt = ptp.tile([128, 4 * QC], BF16, tag="pt", name="pt")
                        nc.scalar.activation(
                            pt[:], psS[:], EXP, scale=0.125 / (QKSCALE * QKSCALE)
                        )
                        if pi == qc:  # diagonal pair: mask the straddling tiles
                            for hl in range(HPC):
                                for ii in range(2):
                                    c0 = QC * (2 * hl + ii) + 128 * ii
                                    nc.vector.tensor_mul(
                                        pt[:, c0 : c0 + 128],
                                        pt[:, c0 : c0 + 128],
                                        tri_sb[:],
                                    )
                        pts.append(pt)
                    # fill the Act-bound late q-chunks with independent PE
                    # work (next batch's qkv chunk / a ready proj half)
                    # between scores and attn@V so exp can catch up
                    if qc >= NQC - len(fillers):
                        fillers[qc - (NQC - len(fillers))]()
                    # attn@V: each psO region's accumulation chain must be
                    # contiguous — a matmul with start=True clears has_written
                    # for its whole PSUM bank, so interleaving region chains
                    # within the bank corrupts accumulation.
                    # psO regions r = 2*u + hl at cols 128r: [q, 64 d | denom]
                    psO = pso.tile([128, 4 * 128], F32, tag="o", name="psO")
                    for u in range(2):
                        for hl in range(HPC):
                            r = 2 * u + hl
                            for kt in range(2 * qc + u + 1):
                                c0 = QC * (2 * hl + (kt & 1)) + 128 * u
                                nc.tensor.matmul(
                                    psO[:, 128 * r : 128 * r + 65],
                                    pts[kt // 2][:, c0 : c0 + 128],
                                    va_t[:, kt, 65 * hl : 65 * (hl + 1)],
                                    start=(kt == 0),
                                    stop=(kt == 2 * qc + u),
                                )
                    # normalize (denominator is psO col 64 of each region),
                    # transpose both heads back to [feature, t]
                    psT = pst.tile([128, QW], BF16, tag="t", name="psT")
                    for u in range(2):
                        zn = znp.tile([128, 128], BF16, tag="zn", name="zn")
                        for hl in range(HPC):
                            r = 2 * u + hl
                            rec = recp.tile([128, 1], F32, tag="rec", name="rec")
                            nc.vector.reciprocal(
                                rec[:], psO[:, 128 * r + 64 : 128 * r + 65]
                            )
                            nc.vector.tensor_scalar_mul(
                                zn[:, 64 * hl : 64 * hl + 64],
                                psO[:, 128 * r : 128 * r + 64],
                                rec[:],
                            )
                        nc.tensor.transpose(
                            psT[:, 128 * u : 128 * (u + 1)], zn[:], ident_sb[:]
                        )
                    # split attn*2^SX into residual-fp8 (lo, hi) for the
                    # DoubleRow proj (DVE: Pool can't read PSUM), then ship
                    # to a2a via the gpsimd SWDGE queue to keep the SP HWDGE
                    # queue free of per-qc head-of-line stalls
                    stg = stagep.tile([128, 2, QW], FP8, tag="stage", name="stg")
                    nc.vector.tensor_copy(stg[:, 1], psT[:])
                    nc.vector.tensor_sub(stg[:, 0], psT[:], stg[:, 1])
                    nc.gpsimd.dma_start(a2a_in[b][qc], stg[:])
                    if dbg is not None and b == 0:
                        nc.gpsimd.dma_start(dbg[qc], stg[:])

            def a2a(b):
                if not sim_mode:
                    nc.gpsimd.collective_compute(
                        "AllToAll",
                        mybir.AluOpType.bypass,
                        replica_groups=groups,
                        ins=[a2a_in[b][:]],
                        outs=[a2a_out[b][:]],
                    )
                # two DMAs for the 8 chunks: [8, 128, 2, 256] -> [128, 8, 2, 256];
                # proj's contraction chains can start on the first half
                r = recvp.tile([128, C // 128, 2, QW], FP8, tag="recv")
                src_ = a2a_out[b][:].rearrange("c p l w -> p c l w")
                nc.sync.dma_start(r[:, 0:4], src_[:, 0:4])
                nc.sync.dma_start(r[:, 4:8], src_[:, 4:8])
                return r

            def proj_quarter(b, recvs, half=None):
                if half in (None, 0):
                    ys = ysp.tile([128, C // 128, QW], F32, tag="ys")
                    proj_quarter.ys = ys
                else:
                    ys = proj_quarter.ys
                obs = range(C // 128) if half is None else range(4 * half, 4 * half + 4)
                for o in obs:
                    psY = psm.tile([128, QW], F32, tag="m", name="psY")
                    fs = slice(o * 128, (o + 1) * 128)
                    for cp in range(4):  # main: w-hi pair x recv-hi pair
                        nc.tensor.matmul(
                            psY[:],
                            wproj_sb[:, cp, 0:2, 0, fs],
                            recvs[:, 2 * cp : 2 * cp + 2, 1, :],
                            start=(cp == 0),
                            stop=False,
                            perf_mode=DR,
                        )
                    for cp in range(4):  # corr: (wh,wl) x (rl,rh)
                        for ci in range(2):
                            nc.tensor.matmul(
                                psY[:],
                                wproj_sb[:, cp, ci, 0:2, fs],
                                recvs[:, 2 * cp + ci, 0:2, :],
                                start=False,
                                stop=(cp == 3 and ci == 1),
                                perf_mode=DR,
                            )
                    # psY = (attn*2^SX)·(w*2^SW) = y*2^(SX+SW)
                    nc.vector.tensor_scalar_mul(ys[:, o], psY[:], 1.0 / QKSCALE)
                    if o % 2 == 1:
                        # fine-grained output DMAs overlap the drain; one big
                        # DMA serializes the quarter behind the last chain
                        nc.sync.dma_start(
                            yT.rearrange("(o p) t -> p o t", p=128)[
                                :, o - 1 : o + 1, QW * b : QW * (b + 1)
                            ],
                            ys[:, o - 1 : o + 1],
                        )

            stage = 0
            pending = []  # (batch, recvs) whose proj is deferred to the tail
            xts = load_x(0)
            tiles = qkv_alloc()
            for tch in range(T // TCH):
                qkv_chunk(tiles, xts, tch)
            for b in range(B):
                if stage >= max_stage:
                    break
                stage += 1
                if b + 1 < B:
                    # prefetch next batch's x before attention's a2a-stage
                    # DMAs so the SP queue doesn't park them behind stages
                    xts_next = load_x(b + 1)
                    tiles_next = qkv_alloc()
                    # next batch's qkv chunks run inside attn(b)'s Act-bound
                    # late q-chunks
                    fillers = [
                        (lambda t: lambda: qkv_chunk(tiles_next, xts_next, t))(t)
                        for t in range(T // TCH)
                    ]
                else:
                    tiles_next = None
                    # last batch: fill with the ready deferred proj halves
                    fillers = [
                        (lambda pb, ph: lambda: proj_quarter(
                            pending[pb][0], pending[pb][1], half=ph
                        ))(pb, ph)
                        for pb in range(2)
                        for ph in range(2)
                    ]
                stage += 1
                attn_batch(b, *tiles, fillers=fillers)
                recvs = a2a(b)
                if b == 0:
                    load_wproj()
                pending.append((b, recvs))
                tiles = tiles_next
            if stage < max_stage:
                for p in pending[2:]:
                    proj_quarter(*p)

    nc.compile()
    return nc


_NC_CACHE = None


def kernel(x: np.ndarray, Wqkv: np.ndarray, Wproj: np.ndarray) -> np.ndarray:
    global _NC_CACHE
    x = np.asarray(x, dtype=np.float32)
    Wqkv = np.asarray(Wqkv, dtype=np.float32)
    Wproj = np.asarray(Wproj, dtype=np.float32)

    bf16 = ml_dtypes.bfloat16
    fp8 = ml_dtypes.float8_e4m3

    def split8(a, s):
        sc = np.float32(2.0**s)
        hi = (a * sc).astype(fp8)
        lo = (a * sc - hi.astype(np.float32)).astype(fp8)
        return hi, lo

    # x8: [128, B, cp, ci, {lo,hi}, T] residual-fp8 of x*2^SX
    xh, xl = split8(x, SX)  # [B, T, C]
    x8 = np.empty((128, B, 4, 2, 2, T), dtype=fp8)
    # c = 128*(2*cp + ci) + p  ->  [B, T, cp, ci, p]
    x8[:, :, :, :, 0, :] = xl.reshape(B, T, 4, 2, 128).transpose(4, 0, 2, 3, 1)
    x8[:, :, :, :, 1, :] = xh.reshape(B, T, 4, 2, 128).transpose(4, 0, 2, 3, 1)

    # wp8: [128, cp, ci, {hi,lo}, 1024] residual-fp8 of Wproj^T * 2^SW
    wpT = np.ascontiguousarray(Wproj.T)  # [1024 c, 1024 f]
    wph, wpl = split8(wpT, SW)
    wp8 = np.empty((128, 4, 2, 2, C), dtype=fp8)
    wp8[:, :, :, 0, :] = wph.reshape(4, 2, 128, C).transpose(2, 0, 1, 3)
    wp8[:, :, :, 1, :] = wpl.reshape(4, 2, 128, C).transpose(2, 0, 1, 3)
    ident = np.eye(128, dtype=np.float32).astype(bf16)
    r = np.arange(128)
    tri = (r[:, None] <= r[None, :]).astype(np.float32).astype(bf16)

    in_maps = []
    for c in range(NCORE):
        rows = slice(c * HPC * D, (c + 1) * HPC * D)  # 128 feature rows
        wq = Wqkv[0 * C :][rows]
        wk = Wqkv[1 * C :][rows]
        wv = Wqkv[2 * C :][rows]
        w_c = np.concatenate([wq, wk, wv], axis=0).T  # [1024 c, 384 f]
        wh, wl = split8(w_c, SW)
        # wq8: [128, cp, ci, {hi,lo}, 384]
        wq8 = np.empty((128, 4, 2, 2, 3 * 128), dtype=fp8)
        wq8[:, :, :, 0, :] = wh.reshape(4, 2, 128, 3 * 128).transpose(2, 0, 1, 3)
        wq8[:, :, :, 1, :] = wl.reshape(4, 2, 128, 3 * 128).transpose(2, 0, 1, 3)
        in_maps.append(
            {
                "x8": x8,
                "wq8": wq8,
                "wp8": wp8,
                "identb": ident,
                "trib": tri,
            }
        )

    if _NC_CACHE is None:
        _NC_CACHE = build_nc()
    res = run_bass_kernel_spmd(_NC_CACHE, in_maps, core_ids=list(range(NCORE)))

    # reassemble: core j returned yT_j [1024, 4*256]; quarter b holds the
    # t-slice [2048*b + 256*j, 2048*b + 256*(j+1)) of the full output
    yT = np.empty((C, BT), dtype=np.float32)
    for j, r_ in enumerate(res.results):
        yTj = r_["yT"]
        for b in range(B):
            yT[:, T * b + QW * j : T * b + QW * (j + 1)] = (
                yTj[:, QW * b : QW * (b + 1)]
            )
    return np.ascontiguousarray(yT.T).reshape(B, T, C)

